# revision 1
# baseline (speedup 1.0000x reference)
"""Causal multi-head attention (B=4, T=2048, C=1024, H=16) on 8 TRN2 cores.

Sharding: batch (4) x head-group (2 groups of 8 heads) -> 8 shards, one per
core. Each core computes QKV projections for its 8 heads, causal flash-style
attention, and a Megatron row-parallel slice of the output projection; the
host sums the two head-group partial outputs per batch element.

All matmul operands are bf16 (PE streams at 2.4 GHz vs fp32r's 1.2), PSUM
accumulation stays f32. K stays resident in SBUF (no DRAM spill).

Attention is emitted as a software pipeline over 128-key slots. Each slot's
S^T tile packs BOTH heads of a pair side by side ([128, 2w]: head hl=0 in
cols 0:w from PE rows 0-63, hl=1 in cols w:2w from rows 64-127): the two
matmuls land in different PSUM banks and different PE row-groups, so they
run concurrently, and one ACT exp covers both heads. PV for slot k is
emitted after S of slot k+1 (lag 1) so the PE isn't gated on ScalarE's exp.
Q/K projection matmul groups for the NEXT head pair are interleaved into the
attention stream as filler to keep the PE dense while ScalarE drains exp;
the output projection for query block tj rides behind the last head pair's
attention on that block. Softmax normalization (1/l) runs entirely off the
PE: DVE copies release PSUM, then fast-reciprocal + GpSimd
partition_broadcast + DVE multiply produce ctx.

Self-contained: hardcodes shapes from the problem spec; no file reads.
"""
import sys
sys.path.insert(0, '/opt/trn_rl_repo')
import numpy as np

B, T, C = 4, 2048, 1024
H, D = 16, 64
N_CORES = 8
HPC = 8        # heads per core
HP = 4         # head pairs per core
KB = 16        # 128-row key tiles per sequence
NQSB = 4       # 512-column query superblocks
CI = 8         # 128-row contraction tiles over C
VW = 66        # V_aug stride per head (64 V + 1 ones + 1 pad)

# Diagonal staircase: block j covers queries [QOFF[j], 512) of the
# superblock (widths 512/384/256/128, exact causal trim at 128 granularity).
# Mask tile layout matches the packed psum tiles: A = j0|j0 (cols 0:1024),
# B = j1,j3|j1,j3 (1024:2048), C = j2 (2048:2304).
QOFF = (0, 128, 256, 384)
DW = tuple(512 - q for q in QOFF)
MW2 = 2304

_CACHE = {}


def build_nc(iters=1):
    import contextlib
    import concourse.tile as tile
    from concourse import bacc, mybir

    F32 = mybir.dt.float32
    BF16 = mybir.dt.bfloat16
    EXP = mybir.ActivationFunctionType.Exp

    nc = bacc.Bacc("TRN2", target_bir_lowering=False, debug=False)

    xT_d = nc.dram_tensor("xT", [C, T], BF16, kind="ExternalInput")
    wqT_d = nc.dram_tensor("wqT", [C, 512], BF16, kind="ExternalInput")
    wkT_d = nc.dram_tensor("wkT", [C, 512], BF16, kind="ExternalInput")
    wvT_d = nc.dram_tensor("wvT", [C, 512], BF16, kind="ExternalInput")
    woT_d = nc.dram_tensor("woT", [512, C], BF16, kind="ExternalInput")
    mask_d = nc.dram_tensor("masks", [128, MW2], BF16, kind="ExternalInput")
    yT_d = nc.dram_tensor("yT", [C, T], BF16, kind="ExternalOutput")

    with tile.TileContext(nc) as tc:
        with contextlib.ExitStack() as es:
            # Pools live outside the For_i loop; tag rotation carries
            # cross-iteration dependencies.
            const = es.enter_context(tc.tile_pool(name="const", bufs=1))
            qtp = es.enter_context(tc.tile_pool(name="qt", bufs=1))
            ktp = es.enter_context(tc.tile_pool(name="kt", bufs=1))
            ctxp = es.enter_context(tc.tile_pool(name="ctx", bufs=1))
            vp = es.enter_context(tc.tile_pool(name="vsb", bufs=1))
            xtp = es.enter_context(tc.tile_pool(name="xt", bufs=1))
            wvp = es.enter_context(tc.tile_pool(name="wv", bufs=1))
            wqp = es.enter_context(tc.tile_pool(name="wq", bufs=2))
            wkp = es.enter_context(tc.tile_pool(name="wk", bufs=2))
            wop = es.enter_context(tc.tile_pool(name="wo", bufs=1))
            maskp = es.enter_context(tc.tile_pool(name="maskp", bufs=1))
            ptp = es.enter_context(tc.tile_pool(name="pt", bufs=8))
            rawp = es.enter_context(tc.tile_pool(name="raw", bufs=3))
            rrowp = es.enter_context(tc.tile_pool(name="rrow", bufs=3))
            bcp = es.enter_context(tc.tile_pool(name="bcp", bufs=3))
            tmpp = es.enter_context(tc.tile_pool(name="tmp", bufs=2))
            yp = es.enter_context(tc.tile_pool(name="y", bufs=3))
            # PSUM: "sp" 2x[128,1024] = 4 banks; "pj" (projections, own
            # tag so filler groups never wait on open pv accumulations)
            # 2x[128,512] = 2; "pv" (attention accumulators) 2x[128,512] = 2.
            sps = es.enter_context(
                tc.tile_pool(name="sps", bufs=2, space="PSUM"))
            pps = es.enter_context(
                tc.tile_pool(name="pps", bufs=2, space="PSUM"))

            def emit(rotated=False):
                ones_f = const.tile([128, 64], F32)
                nc.any.memset(ones_f[:], 1.0)
                ones16_b = const.tile([128, 16], BF16)
                nc.vector.tensor_copy(ones16_b[:], ones_f[:, 0:16])
                mask_sb = maskp.tile([128, MW2], BF16)
                nc.sync.dma_start(mask_sb[:], mask_d.ap())

                qt_sb, kt_sb, ctx_sb, v_sb = [], [], [], []
                for hp in range(HP):
                    qt_sb.append(qtp.tile([128, T], BF16, tag=f"qt{hp}",
                                          name=f"qt{hp}"))
                    kt_sb.append(ktp.tile([128, T], BF16, tag=f"kt{hp}",
                                          name=f"kt{hp}"))
                    ctx_sb.append(ctxp.tile([128, T], BF16, tag=f"ctx{hp}",
                                            name=f"ctx{hp}"))
                for kb in range(KB):
                    v_sb.append(vp.tile([128, HPC * VW], BF16, tag=f"v{kb}",
                                        name=f"v{kb}"))

                xt_sb = []
                for ci in range(CI):
                    t_ = xtp.tile([128, T], BF16, tag=f"xt{ci}")
                    nc.sync.dma_start(t_[:],
                                      xT_d.ap()[ci * 128:(ci + 1) * 128, :])
                    xt_sb.append(t_)
                wo_sb = []
                for hp in range(HP):
                    w_ = wop.tile([128, C], BF16, tag=f"wo{hp}",
                                  name=f"wo{hp}")
                    nc.sync.dma_start(
                        w_[:], woT_d.ap()[hp * 128:(hp + 1) * 128, :])
                    wo_sb.append(w_)

                # ---------------- V projection (resident V_aug) ----------
                wv_sb = []
                for ci in range(CI):
                    t_ = wvp.tile([128, 512], BF16, tag=f"wv{ci}")
                    nc.sync.dma_start(
                        t_[:], wvT_d.ap()[ci * 128:(ci + 1) * 128, :])
                    wv_sb.append(t_)
                def vgroup(ti):
                    ps_ = pps.tile([128, 512], F32, tag="pj", name="vps")
                    for ci in range(CI):
                        nc.tensor.matmul(
                            ps_[:],
                            xt_sb[ci][:, ti * 128:(ti + 1) * 128],
                            wv_sb[ci][:],
                            start=(ci == 0), stop=(ci == CI - 1),
                            skip_group_check=True)
                    sv = v_sb[ti][:].rearrange("p (h w) -> p h w", w=VW)
                    nc.vector.tensor_copy(
                        sv[:, :, 64:66],
                        ones16_b[:].rearrange("p (h w) -> p h w", w=2))
                    nc.vector.tensor_copy(
                        sv[:, :, 0:64],
                        ps_[:].rearrange("p (h w) -> p h w", w=64))

                for ti in range(KB):
                    vgroup(ti)

                def proj_group_fns(hp):
                    """8 filler callables: Q then K psum groups for pair hp."""
                    fsl = slice(hp * 128, (hp + 1) * 128)
                    wq_sb, wk_sb = [], []

                    def load_w():
                        for ci in range(CI):
                            tq = wqp.tile([128, 128], BF16, tag=f"wqs{ci}",
                                          name="wq")
                            nc.sync.dma_start(
                                tq[:],
                                wqT_d.ap()[ci * 128:(ci + 1) * 128, fsl])
                            wq_sb.append(tq)
                            tk = wkp.tile([128, 128], BF16, tag=f"wks{ci}",
                                          name="wk")
                            nc.sync.dma_start(
                                tk[:],
                                wkT_d.ap()[ci * 128:(ci + 1) * 128, fsl])
                            wk_sb.append(tk)

                    def qgroup(tj):
                        tsl = slice(tj * 512, (tj + 1) * 512)
                        ps_ = pps.tile([128, 512], F32, tag="pj", name="qps")
                        for ci in range(CI):
                            nc.tensor.matmul(
                                ps_[:], wq_sb[ci][:], xt_sb[ci][:, tsl],
                                start=(ci == 0), stop=(ci == CI - 1),
                                skip_group_check=True)
                        nc.scalar.copy(qt_sb[hp][:, tsl], ps_[:])

                    def kgroup(tj):
                        tsl = slice(tj * 512, (tj + 1) * 512)
                        ps_ = pps.tile([128, 512], F32, tag="pj", name="kps")
                        for ci in range(CI):
                            nc.tensor.matmul(
                                ps_[:], wk_sb[ci][:], xt_sb[ci][:, tsl],
                                start=(ci == 0), stop=(ci == CI - 1),
                                skip_group_check=True)
                        nc.scalar.copy(kt_sb[hp][:, tsl], ps_[:])

                    fns = [load_w]
                    for tj in range(NQSB):
                        fns.append(lambda tj=tj: qgroup(tj))
                        fns.append(lambda tj=tj: kgroup(tj))
                    return fns

                def attention_pair(hp, qsb, filler):
                    psl = (slice(0, 64), slice(64, 128))
                    vsl = (slice((2 * hp) * VW, (2 * hp) * VW + 65),
                           slice((2 * hp + 1) * VW, (2 * hp + 1) * VW + 65))
                    qbase = qsb * 512
                    n_full = 4 * qsb
                    pv = [pps.tile([128, 512], F32, tag="pv", name="pv0"),
                          pps.tile([128, 512], F32, tag="pv", name="pv1")]
                    first = [True, True]
                    # Slots: n_full full-key tiles plus three diagonal tiles
                    # DA/DB/DC. Every matmul's PSUM output stays inside one
                    # bank (cols 0:512 / 512:1024), and the hl0/hl1 pair of
                    # each block targets different banks so the row-group
                    # concurrency is legal:
                    #   full/DA [128,1024]: hl0 j at 0:512, hl1 j at 512:1024
                    #   DB [128,1024]: j1 h0 0:384, j3 h0 384:512,
                    #                  j1 h1 512:896, j3 h1 896:1024
                    #   DC [128,768]:  j2 h0 0:256, j2 h1 512:768 (gap unread)
                    def smm(sp, c0, c1, hl, kb, qoff):
                        nc.tensor.matmul(
                            sp[:, c0:c1],
                            kt_sb[hp][psl[hl], kb * 128:(kb + 1) * 128],
                            qt_sb[hp][psl[hl], qbase + qoff:qbase + 512],
                            start=True, stop=True, skip_group_check=True)

                    def pvmm(hl, kb, pt_ap, qoff, stop):
                        nc.tensor.matmul(
                            pv[hl][0:65, qoff:512],
                            v_sb[kb][:, vsl[hl]], pt_ap,
                            start=first[hl], stop=stop,
                            skip_group_check=True)
                        first[hl] = False

                    nslots = n_full + 3
                    pts = {}

                    def emit_S(idx):
                        if idx < n_full + 1:          # full tile or DA
                            kb = idx
                            sp = sps.tile([128, 1024], F32, tag="sp",
                                          name="sp")
                            smm(sp, 0, 512, 0, kb, 0)
                            smm(sp, 512, 1024, 1, kb, 0)
                            pt = ptp.tile([128, 1024], BF16, tag="pt",
                                          name="pt")
                            nc.scalar.activation(pt[:], sp[:], EXP,
                                                 scale=0.125)
                            if idx == n_full:         # DA: j0 triangle mask
                                nc.vector.tensor_mul(pt[:], pt[:],
                                                     mask_sb[:, 0:1024])
                        elif idx == n_full + 1:       # DB: j1 + j3
                            sp = sps.tile([128, 1024], F32, tag="sp",
                                          name="sp")
                            smm(sp, 0, 384, 0, n_full + 1, 128)
                            smm(sp, 512, 896, 1, n_full + 1, 128)
                            smm(sp, 384, 512, 0, n_full + 3, 384)
                            smm(sp, 896, 1024, 1, n_full + 3, 384)
                            pt = ptp.tile([128, 1024], BF16, tag="pt",
                                          name="pt")
                            nc.scalar.activation(pt[:], sp[:], EXP,
                                                 scale=0.125)
                            nc.vector.tensor_mul(pt[:], pt[:],
                                                 mask_sb[:, 1024:2048])
                        else:                         # DC: j2
                            sp = sps.tile([128, 768], F32, tag="sp",
                                          name="sp")
                            smm(sp, 0, 256, 0, n_full + 2, 256)
                            smm(sp, 512, 768, 1, n_full + 2, 256)
                            pt = ptp.tile([128, 768], BF16, tag="pt",
                                          name="pt")
                            nc.scalar.activation(pt[:, 0:256], sp[:, 0:256],
                                                 EXP, scale=0.125)
                            nc.scalar.activation(pt[:, 512:768],
                                                 sp[:, 512:768],
                                                 EXP, scale=0.125)
                            nc.vector.tensor_mul(pt[:, 0:256], pt[:, 0:256],
                                                 mask_sb[:, 2048:2304])
                            nc.vector.tensor_mul(pt[:, 512:768],
                                                 pt[:, 512:768],
                                                 mask_sb[:, 2048:2304])
                        pts[idx] = pt

                    def emit_PV(idx):
                        pt = pts.pop(idx)
                        if idx < n_full + 1:
                            pvmm(0, idx, pt[:, 0:512], 0, False)
                            pvmm(1, idx, pt[:, 512:1024], 0, False)
                        elif idx == n_full + 1:
                            pvmm(0, n_full + 1, pt[:, 0:384], 128, False)
                            pvmm(1, n_full + 1, pt[:, 512:896], 128, False)
                            pvmm(0, n_full + 3, pt[:, 384:512], 384, False)
                            pvmm(1, n_full + 3, pt[:, 896:1024], 384, False)
                        else:                         # DC last: stop
                            pvmm(0, n_full + 2, pt[:, 0:256], 256, True)
                            pvmm(1, n_full + 2, pt[:, 512:768], 256, True)

                    # full slots: PV lag-1. Diagonal slots: emit all three
                    # S/exp/mask stages first, then their PVs — the DVE
                    # mask multiplies get ~3 slots of slack instead of 1.
                    for idx in range(nslots):
                        emit_S(idx)
                        if 1 <= idx <= n_full:
                            emit_PV(idx - 1)
                        filler()
                    for idx in range(n_full, nslots):
                        emit_PV(idx)

                    # normalize: ctx = pv[0:64] * (1 / pv[64]); off the PE
                    # queue. The two DVE copies release pv's PSUM early; the
                    # recip/broadcast/mul chain then runs off SBUF. Custom
                    # DVE ops and partition_broadcast need base partition 0,
                    # so the l row is copied 64 -> 0 first.
                    # Stage-major across the two heads: all DVE copies
                    # and recips first, then both GpSimd broadcasts, then
                    # both multiplies — the in-order DVE queue works on one
                    # head's copies while the other head's broadcast is in
                    # flight, instead of stalling at the mul (with the next
                    # block's mask muls queued behind the stall). Two tiles
                    # per tag are live at once; bufs=3 pools cover that.
                    raws, rfs, bcs = [], [], []
                    for hl in range(2):
                        lrow = rawp.tile([1, 512], F32, tag="lrow",
                                         name="lrow")
                        nc.vector.tensor_copy(lrow[0:1, :],
                                              pv[hl][64:65, :])
                        raw = rawp.tile([64, 512], F32, tag="raw",
                                        name="raw")
                        nc.vector.tensor_copy(raw[:], pv[hl][0:64, :])
                        rf = rrowp.tile([1, 512], F32, tag="rf", name="rf")
                        nc.vector.reciprocal_approx_fast(
                            rf[0:1, :], lrow[0:1, :])
                        raws.append(raw)
                        rfs.append(rf)
                    for hl in range(2):
                        bcast = bcp.tile([64, 512], F32, tag="bc",
                                         name="bcast")
                        nc.gpsimd.partition_broadcast(bcast[0:64, :],
                                                      rfs[hl][0:1, :])
                        bcs.append(bcast)
                    nc.vector.tensor_mul(
                        ctx_sb[hp][0:64, qbase:qbase + 512],
                        raws[0][:], bcs[0][:])
                    tmp = tmpp.tile([64, 512], BF16, name="tmp")
                    nc.vector.tensor_mul(tmp[:], raws[1][:], bcs[1][:])
                    nc.vector.tensor_copy(
                        ctx_sb[hp][64:128, qbase:qbase + 512],
                        tmp[:])

                def project_out_fns(tj, tail=False):
                    """8 per-oi output-projection callables for block tj.
                    The output bias is added on the host during the gather.
                    In the tail (after all attention), alternate psum tags
                    and copy engines so the drain isn't serialized on one
                    engine or one psum rotation."""
                    tsl = slice(tj * 512, (tj + 1) * 512)

                    def ogroup(oi):
                        ps_ = pps.tile([128, 512], F32, tag="pj",
                                       name="yacc")
                        osl = slice(oi * 128, (oi + 1) * 128)
                        for hp in range(HP):
                            nc.tensor.matmul(
                                ps_[:], wo_sb[hp][:, osl],
                                ctx_sb[hp][:, tsl],
                                start=(hp == 0), stop=(hp == HP - 1),
                                skip_group_check=True)
                        y_ = yp.tile([128, 512], BF16, name="y_")
                        # DVE, not ScalarE: the gap list is exp-paced, and
                        # stage-major normalize freed DVE queue headroom
                        nc.vector.tensor_copy(y_[:], ps_[:])
                        nc.sync.dma_start(yT_d.ap()[osl, tsl], y_[:])
                    return [lambda oi=oi: ogroup(oi) for oi in range(8)]

                # main schedule: V proj + QK(hp0) upfront; per head pair,
                # attention with the next pair's QK groups interleaved as
                # filler (one group every 5th slot). The last pair instead
                # fills with the previous query block's output projection
                # (one group per slot). In the rotated (For_i) body, the
                # final query block's output projection runs here — after
                # V-proj, on the PREVIOUS iteration's ctx (same SBUF slots
                # via tag rotation) — so its DVE normalize chain drains
                # under V-proj instead of stalling the PE at the tail. An
                # epilogue iteration after the loop emits the true final
                # values; iteration 0's garbage q3 write is overwritten.
                if rotated:
                    for fn in project_out_fns(NQSB - 1, tail=True):
                        fn()
                for fn in proj_group_fns(0):
                    fn()
                for hp in range(HP):
                    last = hp == HP - 1
                    pending = [] if last else proj_group_fns(hp + 1)
                    state = {"n": 0}

                    def filler():
                        state["n"] += 1
                        if pending and (last or state["n"] % 5 == 2):
                            pending.pop(0)()
                    for qsb in range(NQSB):
                        if last and qsb > 0:
                            pending.extend(project_out_fns(qsb - 1))
                        attention_pair(hp, qsb, filler)
                    while pending:
                        pending.pop(0)()
                    if last and not rotated:
                        for fn in project_out_fns(NQSB - 1, tail=True):
                            fn()

            if iters == 1:
                emit()
            else:
                for hp in range(HP):
                    ci_ = ctxp.tile([128, T], BF16, tag=f"ctx{hp}",
                                    name=f"ctxinit{hp}")
                    nc.any.memset(ci_[:], 0.0)
                with tc.For_i(0, iters, 1):
                    emit(rotated=True)
                emit()
    nc.compile()
    return nc


def make_masks():
    """Masks [128, MW2]: causal keep-bits for key row k = 128*j + k_local vs
    query q, laid out to match the packed psum tiles (A = j0|j0,
    B = j1,j3|j1,j3, C = j2)."""
    def blk(j):
        q = np.arange(QOFF[j], 512)[None, :]
        k = np.arange(128)[:, None]
        return (q >= 128 * j + k).astype(np.float32)
    b0, b1, b2, b3 = blk(0), blk(1), blk(2), blk(3)
    return np.concatenate([b0, b0, b1, b3, b1, b3, b2], axis=1)


def shard_inputs(x, w_qkv, w_out, b_out):
    """Full inputs -> list of 8 per-core input dicts (weights/x in bf16)."""
    import ml_dtypes
    bf16 = ml_dtypes.bfloat16
    x = np.asarray(x, dtype=np.float32)
    w_qkv = np.asarray(w_qkv, dtype=np.float32)
    w_out = np.asarray(w_out, dtype=np.float32)
    b_out = np.asarray(b_out, dtype=np.float32)
    masks = make_masks().astype(bf16)
    in_maps = []
    for c in range(N_CORES):
        b, hg = c // 2, c % 2
        h0 = hg * HPC
        csl = slice(h0 * D, (h0 + HPC) * D)
        im = {
            "xT": np.ascontiguousarray(x[b].T).astype(bf16),
            "wqT": np.ascontiguousarray(w_qkv[0 * C:1 * C][csl].T).astype(bf16),
            "wkT": np.ascontiguousarray(w_qkv[1 * C:2 * C][csl].T).astype(bf16),
            "wvT": np.ascontiguousarray(w_qkv[2 * C:3 * C][csl].T).astype(bf16),
            "woT": np.ascontiguousarray(w_out[:, csl].T).astype(bf16),
            "masks": masks,
        }
        in_maps.append(im)
    return in_maps


def gather_outputs(results, b_out=None):
    """8 per-core {'yT': [C,T]} -> full [B,T,C] (+ output bias)."""
    y = np.empty((B, T, C), np.float32)
    for b in range(B):
        acc = (results[2 * b]["yT"].astype(np.float32)
               + results[2 * b + 1]["yT"].astype(np.float32))
        y[b] = acc.T
    if b_out is not None:
        y += np.asarray(b_out, dtype=np.float32)[None, None, :]
    return y


def kernel(**inputs):
    from concourse.bass_utils import run_bass_kernel_spmd
    if "nc" not in _CACHE:
        _CACHE["nc"] = build_nc()
    nc = _CACHE["nc"]
    in_maps = shard_inputs(inputs["x"], inputs["w_qkv"],
                           inputs["w_out"], inputs["b_out"])
    res = run_bass_kernel_spmd(nc, in_maps, list(range(N_CORES)))
    return gather_outputs(res.results, inputs["b_out"])



# revision 22
# speedup vs baseline: 1.1010x; 1.1010x over previous
"""Causal multi-head attention (B=4, T=2048, C=1024, H=16) on 8 TRN2 cores.

Sharding: batch (4) x head-group (2 groups of 8 heads) -> 8 shards, one per
core. Each core computes QKV projections for its 8 heads, causal flash-style
attention, and a Megatron row-parallel slice of the output projection; the
host sums the two head-group partial outputs per batch element.

All matmul operands are bf16 (PE streams at 2.4 GHz vs fp32r's 1.2), PSUM
accumulation stays f32. K stays resident in SBUF (no DRAM spill).

Attention is emitted as a software pipeline over 128-key slots. Each slot's
S^T tile packs BOTH heads of a pair side by side ([128, 2w]: head hl=0 in
cols 0:w from PE rows 0-63, hl=1 in cols w:2w from rows 64-127): the two
matmuls land in different PSUM banks and different PE row-groups, so they
run concurrently, and one ACT exp covers both heads. PV for slot k is
emitted after S of slot k+1 (lag 1) so the PE isn't gated on ScalarE's exp.
Q/K projection matmul groups for the NEXT head pair are interleaved into the
attention stream as filler to keep the PE dense while ScalarE drains exp;
the output projection for query block tj rides behind the last head pair's
attention on that block. Softmax normalization (1/l) runs entirely off the
PE: DVE copies release PSUM, then fast-reciprocal + GpSimd
partition_broadcast + DVE multiply produce ctx.

Self-contained: hardcodes shapes from the problem spec; no file reads.
"""
import sys
sys.path.insert(0, '/opt/trn_rl_repo')
import numpy as np

B, T, C = 4, 2048, 1024
H, D = 16, 64
N_CORES = 8
HPC = 8        # heads per core
HP = 4         # head pairs per core
KB = 16        # 128-row key tiles per sequence
NQSB = 4       # 512-column query superblocks
CI = 8         # 128-row contraction tiles over C
VW = 66        # V_aug stride per head (64 V + 1 ones + 1 pad)

# Diagonal staircase: block j covers queries [QOFF[j], 512) of the
# superblock (widths 512/384/256/128, exact causal trim at 128 granularity).
# Mask tile layout matches the packed psum tiles: A = j0|j0 (cols 0:1024),
# B = j1,j3|j1,j3 (1024:2048), C = j2,gap,j2 (2048:2816; the duplicate at
# stride 512 lets one strided-AP DVE mul mask both heads' DC regions).
QOFF = (0, 128, 256, 384)
DW = tuple(512 - q for q in QOFF)
MW2 = 2816

_CACHE = {}


def build_nc(iters=1):
    import contextlib
    import concourse.tile as tile
    from concourse import bacc, mybir

    F32 = mybir.dt.float32
    BF16 = mybir.dt.bfloat16
    EXP = mybir.ActivationFunctionType.Exp

    nc = bacc.Bacc("TRN2", target_bir_lowering=False, debug=False)

    xT_d = nc.dram_tensor("xT", [C, T], BF16, kind="ExternalInput")
    wqT_d = nc.dram_tensor("wqT", [C, 512], BF16, kind="ExternalInput")
    wkT_d = nc.dram_tensor("wkT", [C, 512], BF16, kind="ExternalInput")
    wvT_d = nc.dram_tensor("wvT", [C, 512], BF16, kind="ExternalInput")
    woT_d = nc.dram_tensor("woT", [512, C], BF16, kind="ExternalInput")
    mask_d = nc.dram_tensor("masks", [128, MW2], BF16, kind="ExternalInput")
    yT_d = nc.dram_tensor("yT", [C, T], BF16, kind="ExternalOutput")

    with tile.TileContext(nc) as tc:
        with contextlib.ExitStack() as es:
            # Pools live outside the For_i loop; tag rotation carries
            # cross-iteration dependencies.
            const = es.enter_context(tc.tile_pool(name="const", bufs=1))
            qtp = es.enter_context(tc.tile_pool(name="qt", bufs=1))
            ktp = es.enter_context(tc.tile_pool(name="kt", bufs=1))
            ctxp = es.enter_context(tc.tile_pool(name="ctx", bufs=1))
            vp = es.enter_context(tc.tile_pool(name="vsb", bufs=1))
            xtp = es.enter_context(tc.tile_pool(name="xt", bufs=1))
            wvp = es.enter_context(tc.tile_pool(name="wv", bufs=1))
            wqp = es.enter_context(tc.tile_pool(name="wq", bufs=1))
            wkp = es.enter_context(tc.tile_pool(name="wk", bufs=1))
            wop = es.enter_context(tc.tile_pool(name="wo", bufs=1))
            maskp = es.enter_context(tc.tile_pool(name="maskp", bufs=1))
            ptp = es.enter_context(tc.tile_pool(name="pt", bufs=8))
            rawp = es.enter_context(tc.tile_pool(name="raw", bufs=3))
            rrowp = es.enter_context(tc.tile_pool(name="rrow", bufs=3))
            bcp = es.enter_context(tc.tile_pool(name="bcp", bufs=3))
            tmpp = es.enter_context(tc.tile_pool(name="tmp", bufs=2))
            yp = es.enter_context(tc.tile_pool(name="y", bufs=3))
            # PSUM: "sp" 2x[128,1024] = 4 banks; "pj" (projections, own
            # tag so filler groups never wait on open pv accumulations)
            # 2x[128,512] = 2; "pv" (attention accumulators) 2x[128,512] = 2.
            sps = es.enter_context(
                tc.tile_pool(name="sps", bufs=2, space="PSUM"))
            pps = es.enter_context(
                tc.tile_pool(name="pps", bufs=2, space="PSUM"))

            # Constants (ones, causal mask): loaded ONCE - they are internal
            # constants, not kernel inputs, so they sit outside the per-
            # iteration input reload.
            ones_f = const.tile([128, 64], F32)
            nc.any.memset(ones_f[:], 1.0)
            ones16_b = const.tile([128, 16], BF16)
            nc.vector.tensor_copy(ones16_b[:], ones_f[:, 0:16])
            mask_sb = maskp.tile([128, MW2], BF16, name="mask")
            nc.sync.dma_start(mask_sb[:], mask_d.ap())

            def alloc_inputs():
                """Input tiles by tag (bufs=1: fixed buffers). DMA issue is
                separate so the loop can prefetch next-iter inputs at body
                tail while this body computes on last-tail's data."""
                xt_sb = [xtp.tile([128, T], BF16, tag=f"xt{ci}",
                                  name=f"xt{ci}") for ci in range(CI)]
                wo_sb = [wop.tile([128, C], BF16, tag=f"wo{hp}",
                                  name=f"wo{hp}") for hp in range(HP)]
                wv_sb = [wvp.tile([128, 512], BF16, tag=f"wv{ci}",
                                  name=f"wv{ci}") for ci in range(CI)]
                wq0_sb = [wqp.tile([128, 128], BF16, tag=f"wqs{ci}",
                                   name="wq") for ci in range(CI)]
                wk0_sb = [wkp.tile([128, 128], BF16, tag=f"wks{ci}",
                                   name="wk") for ci in range(CI)]
                return xt_sb, wo_sb, wv_sb, wq0_sb, wk0_sb

            def load_inputs(tiles):
                # wo first: the rotated body's top oproj needs it; x/wv last
                # (V-proj follows the ~7us oproj, adding DMA slack).
                xt_sb, wo_sb, wv_sb, wq0_sb, wk0_sb = tiles
                for hp in range(HP):
                    nc.sync.dma_start(
                        wo_sb[hp][:], woT_d.ap()[hp * 128:(hp + 1) * 128, :])
                for ci in range(CI):
                    nc.sync.dma_start(xt_sb[ci][:],
                                      xT_d.ap()[ci * 128:(ci + 1) * 128, :])
                for ci in range(CI):
                    nc.sync.dma_start(
                        wv_sb[ci][:], wvT_d.ap()[ci * 128:(ci + 1) * 128, :])
                for ci in range(CI):
                    nc.sync.dma_start(wq0_sb[ci][:],
                                      wqT_d.ap()[ci * 128:(ci + 1) * 128,
                                                 0:128])
                    nc.sync.dma_start(wk0_sb[ci][:],
                                      wkT_d.ap()[ci * 128:(ci + 1) * 128,
                                                 0:128])

            def emit(rotated=False, loads="top"):
                in_tiles = alloc_inputs()
                xt_sb, wo_sb, wv_sb, wq0_sb, wk0_sb = in_tiles
                if loads == "top":
                    load_inputs(in_tiles)

                qt_sb, kt_sb, ctx_sb, v_sb = [], [], [], []
                for hp in range(HP):
                    qt_sb.append(qtp.tile([128, T], BF16, tag=f"qt{hp}",
                                          name=f"qt{hp}"))
                    kt_sb.append(ktp.tile([128, T], BF16, tag=f"kt{hp}",
                                          name=f"kt{hp}"))
                    ctx_sb.append(ctxp.tile([128, T], BF16, tag=f"ctx{hp}",
                                            name=f"ctx{hp}"))
                for kb in range(KB):
                    v_sb.append(vp.tile([128, HPC * VW], BF16, tag=f"v{kb}",
                                        name=f"v{kb}"))
                def vgroup(ti):
                    ps_ = pps.tile([128, 512], F32, tag="pj", name="vps")
                    for ci in range(CI):
                        nc.tensor.matmul(
                            ps_[:],
                            xt_sb[ci][:, ti * 128:(ti + 1) * 128],
                            wv_sb[ci][:],
                            start=(ci == 0), stop=(ci == CI - 1),
                            skip_group_check=True)
                    sv = v_sb[ti][:].rearrange("p (h w) -> p h w", w=VW)
                    nc.vector.tensor_copy(
                        sv[:, :, 64:66],
                        ones16_b[:].rearrange("p (h w) -> p h w", w=2))
                    nc.vector.tensor_copy(
                        sv[:, :, 0:64],
                        ps_[:].rearrange("p (h w) -> p h w", w=64))

                def proj_group_fns(hp, preloaded=None):
                    """Filler callables: Q then K psum groups for pair hp.
                    preloaded=(wq_sb, wk_sb) skips the load (hp0: weights
                    DMA'd at the previous body's tail / the prologue)."""
                    fsl = slice(hp * 128, (hp + 1) * 128)
                    if preloaded is not None:
                        wq_sb, wk_sb = list(preloaded[0]), list(preloaded[1])
                    else:
                        wq_sb, wk_sb = [], []

                    def load_w():
                        for ci in range(CI):
                            tq = wqp.tile([128, 128], BF16, tag=f"wqs{ci}",
                                          name="wq")
                            nc.sync.dma_start(
                                tq[:],
                                wqT_d.ap()[ci * 128:(ci + 1) * 128, fsl])
                            wq_sb.append(tq)
                            tk = wkp.tile([128, 128], BF16, tag=f"wks{ci}",
                                          name="wk")
                            nc.sync.dma_start(
                                tk[:],
                                wkT_d.ap()[ci * 128:(ci + 1) * 128, fsl])
                            wk_sb.append(tk)

                    def qgroup(tj):
                        tsl = slice(tj * 512, (tj + 1) * 512)
                        ps_ = pps.tile([128, 512], F32, tag="pj", name="qps")
                        for ci in range(CI):
                            nc.tensor.matmul(
                                ps_[:], wq_sb[ci][:], xt_sb[ci][:, tsl],
                                start=(ci == 0), stop=(ci == CI - 1),
                                skip_group_check=True)
                        nc.gpsimd.tensor_copy(qt_sb[hp][:, tsl], ps_[:])

                    def kgroup(tj):
                        tsl = slice(tj * 512, (tj + 1) * 512)
                        ps_ = pps.tile([128, 512], F32, tag="pj", name="kps")
                        for ci in range(CI):
                            nc.tensor.matmul(
                                ps_[:], wk_sb[ci][:], xt_sb[ci][:, tsl],
                                start=(ci == 0), stop=(ci == CI - 1),
                                skip_group_check=True)
                        nc.gpsimd.tensor_copy(kt_sb[hp][:, tsl], ps_[:])

                    fns = [] if preloaded is not None else [load_w]
                    for tj in range(NQSB):
                        fns.append(lambda tj=tj: qgroup(tj))
                        fns.append(lambda tj=tj: kgroup(tj))
                    return fns

                def attention_pair(hp, qsb, filler):
                    psl = (slice(0, 64), slice(64, 128))
                    vsl = (slice((2 * hp) * VW, (2 * hp) * VW + 65),
                           slice((2 * hp + 1) * VW, (2 * hp + 1) * VW + 65))
                    qbase = qsb * 512
                    n_full = 4 * qsb
                    pv = [pps.tile([128, 512], F32, tag="pv", name="pv0"),
                          pps.tile([128, 512], F32, tag="pv", name="pv1")]
                    first = [True, True]
                    # Slots: n_full full-key tiles plus three diagonal tiles
                    # DA/DB/DC. Every matmul's PSUM output stays inside one
                    # bank (cols 0:512 / 512:1024), and the hl0/hl1 pair of
                    # each block targets different banks so the row-group
                    # concurrency is legal:
                    #   full/DA [128,1024]: hl0 j at 0:512, hl1 j at 512:1024
                    #   DB [128,1024]: j1 h0 0:384, j3 h0 384:512,
                    #                  j1 h1 512:896, j3 h1 896:1024
                    #   DC [128,768]:  j2 h0 0:256, j2 h1 512:768 (gap unread)
                    def smm(sp, c0, c1, hl, kb, qoff):
                        nc.tensor.matmul(
                            sp[:, c0:c1],
                            kt_sb[hp][psl[hl], kb * 128:(kb + 1) * 128],
                            qt_sb[hp][psl[hl], qbase + qoff:qbase + 512],
                            start=True, stop=True, skip_group_check=True)

                    def pvmm(hl, kb, pt_ap, qoff, stop):
                        nc.tensor.matmul(
                            pv[hl][0:65, qoff:512],
                            v_sb[kb][:, vsl[hl]], pt_ap,
                            start=first[hl], stop=stop,
                            skip_group_check=True)
                        first[hl] = False

                    nslots = n_full + 3
                    pts = {}

                    def emit_S(idx):
                        if idx < n_full + 1:          # full tile or DA
                            kb = idx
                            sp = sps.tile([128, 1024], F32, tag="sp",
                                          name="sp")
                            smm(sp, 0, 512, 0, kb, 0)
                            smm(sp, 512, 1024, 1, kb, 0)
                            pt = ptp.tile([128, 1024], BF16, tag="pt",
                                          name="pt")
                            nc.scalar.activation(pt[:], sp[:], EXP,
                                                 scale=0.125)
                            if idx == n_full:         # DA: j0 triangle mask
                                nc.vector.tensor_mul(pt[:], pt[:],
                                                     mask_sb[:, 0:1024])
                        elif idx == n_full + 1:       # DB: j1 + j3
                            sp = sps.tile([128, 1024], F32, tag="sp",
                                          name="sp")
                            smm(sp, 0, 384, 0, n_full + 1, 128)
                            smm(sp, 512, 896, 1, n_full + 1, 128)
                            smm(sp, 384, 512, 0, n_full + 3, 384)
                            smm(sp, 896, 1024, 1, n_full + 3, 384)
                            pt = ptp.tile([128, 1024], BF16, tag="pt",
                                          name="pt")
                            nc.scalar.activation(pt[:], sp[:], EXP,
                                                 scale=0.125)
                            nc.vector.tensor_mul(pt[:], pt[:],
                                                 mask_sb[:, 1024:2048])
                        else:                         # DC: j2
                            sp = sps.tile([128, 768], F32, tag="sp",
                                          name="sp")
                            smm(sp, 0, 256, 0, n_full + 2, 256)
                            smm(sp, 512, 768, 1, n_full + 2, 256)
                            pt = ptp.tile([128, 768], BF16, tag="pt",
                                          name="pt")
                            # one strided-AP instruction covers both heads'
                            # 256-wide regions (cols 0:256 and 512:768)
                            ptv = pt[:].rearrange(
                                "p (a b) -> p a b", b=256)[:, 0:3:2, :]
                            spv = sp[:].rearrange(
                                "p (a b) -> p a b", b=256)[:, 0:3:2, :]
                            mkv = mask_sb[:, 2048:2816].rearrange(
                                "p (a b) -> p a b", b=256)[:, 0:3:2, :]
                            nc.scalar.activation(ptv, spv, EXP, scale=0.125)
                            nc.vector.tensor_mul(ptv, ptv, mkv)
                        pts[idx] = pt

                    def emit_PV(idx):
                        pt = pts.pop(idx)
                        if idx < n_full + 1:
                            pvmm(0, idx, pt[:, 0:512], 0, False)
                            pvmm(1, idx, pt[:, 512:1024], 0, False)
                        elif idx == n_full + 1:
                            pvmm(0, n_full + 1, pt[:, 0:384], 128, False)
                            pvmm(1, n_full + 1, pt[:, 512:896], 128, False)
                            pvmm(0, n_full + 3, pt[:, 384:512], 384, False)
                            pvmm(1, n_full + 3, pt[:, 896:1024], 384, False)
                        else:                         # DC last: stop
                            pvmm(0, n_full + 2, pt[:, 0:256], 256, True)
                            pvmm(1, n_full + 2, pt[:, 512:768], 256, True)

                    # full slots: PV lag-1. Diagonal slots: emit all three
                    # S/exp/mask stages first, then their PVs — the DVE
                    # mask multiplies get ~3 slots of slack instead of 1.
                    for idx in range(nslots):
                        emit_S(idx)
                        if 1 <= idx <= n_full:
                            emit_PV(idx - 1)
                        filler()
                    for idx in range(n_full, nslots):
                        emit_PV(idx)

                    # normalize: ctx = pv[0:64] * (1 / pv[64]); off the PE
                    # queue. The two DVE copies release pv's PSUM early; the
                    # recip/broadcast/mul chain then runs off SBUF. Custom
                    # DVE ops and partition_broadcast need base partition 0,
                    # so the l row is copied 64 -> 0 first.
                    # Stage-major across the two heads: all DVE copies
                    # and recips first, then both GpSimd broadcasts, then
                    # both multiplies — the in-order DVE queue works on one
                    # head's copies while the other head's broadcast is in
                    # flight, instead of stalling at the mul (with the next
                    # block's mask muls queued behind the stall). Two tiles
                    # per tag are live at once; bufs=3 pools cover that.
                    raws, rfs, bcs = [], [], []
                    lrows = []
                    for hl in range(2):
                        lrow = rawp.tile([1, 512], F32, tag="lrow",
                                         name="lrow")
                        raw = rawp.tile([64, 512], F32, tag="raw",
                                        name="raw")
                        # split the psum-release copies across ACT/DVE so
                        # both pv banks free in ~half the time at block ends
                        if hl == 0:
                            nc.scalar.copy(lrow[0:1, :], pv[hl][64:65, :])
                            nc.scalar.copy(raw[:], pv[hl][0:64, :])
                        else:
                            nc.vector.tensor_copy(lrow[0:1, :],
                                                  pv[hl][64:65, :])
                            nc.vector.tensor_copy(raw[:], pv[hl][0:64, :])
                        lrows.append(lrow)
                        raws.append(raw)
                    for hl in range(2):
                        rf = rrowp.tile([1, 512], F32, tag="rf", name="rf")
                        nc.vector.reciprocal_approx_fast(
                            rf[0:1, :], lrows[hl][0:1, :])
                        rfs.append(rf)
                    for hl in range(2):
                        bcast = bcp.tile([64, 512], F32, tag="bc",
                                         name="bcast")
                        nc.gpsimd.partition_broadcast(bcast[0:64, :],
                                                      rfs[hl][0:1, :])
                        bcs.append(bcast)
                    nc.vector.tensor_mul(
                        ctx_sb[hp][0:64, qbase:qbase + 512],
                        raws[0][:], bcs[0][:])
                    tmp = tmpp.tile([64, 512], BF16, name="tmp")
                    nc.vector.tensor_mul(tmp[:], raws[1][:], bcs[1][:])
                    nc.vector.tensor_copy(
                        ctx_sb[hp][64:128, qbase:qbase + 512],
                        tmp[:])

                def project_out_fns(tj, tail=False):
                    """8 per-oi output-projection callables for block tj.
                    The output bias is added on the host during the gather.
                    In the tail (after all attention), alternate psum tags
                    and copy engines so the drain isn't serialized on one
                    engine or one psum rotation."""
                    tsl = slice(tj * 512, (tj + 1) * 512)

                    def ogroup(oi):
                        ps_ = pps.tile([128, 512], F32, tag="pj",
                                       name="yacc")
                        osl = slice(oi * 128, (oi + 1) * 128)
                        for hp in range(HP):
                            nc.tensor.matmul(
                                ps_[:], wo_sb[hp][:, osl],
                                ctx_sb[hp][:, tsl],
                                start=(hp == 0), stop=(hp == HP - 1),
                                skip_group_check=True)
                        y_ = yp.tile([128, 512], BF16, name="y_")
                        # DVE, not ScalarE: the gap list is exp-paced
                        # (GpSimd can't read PSUM on TRN2)
                        nc.vector.tensor_copy(y_[:], ps_[:])
                        nc.sync.dma_start(yT_d.ap()[osl, tsl], y_[:])
                    return [lambda oi=oi: ogroup(oi) for oi in range(8)]

                # main schedule: V proj + QK(hp0) upfront; per head pair,
                # attention with the next pair's QK groups interleaved as
                # filler (one group every 5th slot). The last pair instead
                # fills with the previous query block's output projection
                # (one group per slot). In the rotated (For_i) body, the
                # final query block's output projection runs here — after
                # V-proj, on the PREVIOUS iteration's ctx (same SBUF slots
                # via tag rotation) — so its DVE normalize chain drains
                # under V-proj instead of stalling the PE at the tail. An
                # epilogue iteration after the loop emits the true final
                # values; iteration 0's garbage q3 write is overwritten.
                # rotated: previous iteration's final oproj runs FIRST (needs
                # only wo + prev ctx), covering the x/wv prefetch still in
                # flight; V-proj follows.
                if rotated:
                    for fn in project_out_fns(NQSB - 1, tail=True):
                        fn()
                for ti in range(KB):
                    vgroup(ti)
                for fn in proj_group_fns(0, preloaded=(wq0_sb, wk0_sb)):
                    fn()
                for hp in range(HP):
                    last = hp == HP - 1
                    pending = [] if last else proj_group_fns(hp + 1)
                    state = {"n": 0}

                    def filler():
                        state["n"] += 1
                        if pending and (last or state["n"] % 5 == 2):
                            pending.pop(0)()
                    for qsb in range(NQSB):
                        if last and qsb > 0:
                            pending.extend(project_out_fns(qsb - 1))
                        attention_pair(hp, qsb, filler)
                    while pending:
                        pending.pop(0)()
                    if last and not rotated:
                        for fn in project_out_fns(NQSB - 1, tail=True):
                            fn()
                if loads == "tail":
                    load_inputs(in_tiles)

            if iters == 1:
                emit()
            else:
                for hp in range(HP):
                    ci_ = ctxp.tile([128, T], BF16, tag=f"ctx{hp}",
                                    name=f"ctxinit{hp}")
                    nc.any.memset(ci_[:], 0.0)
                load_inputs(alloc_inputs())
                with tc.For_i(0, iters, 1):
                    emit(rotated=True, loads="tail")
                emit(loads="top")
    nc.compile()
    return nc


def make_masks():
    """Masks [128, MW2]: causal keep-bits for key row k = 128*j + k_local vs
    query q, laid out to match the packed psum tiles (A = j0|j0,
    B = j1,j3|j1,j3, C = j2)."""
    def blk(j):
        q = np.arange(QOFF[j], 512)[None, :]
        k = np.arange(128)[:, None]
        return (q >= 128 * j + k).astype(np.float32)
    b0, b1, b2, b3 = blk(0), blk(1), blk(2), blk(3)
    gap = np.zeros((128, 256), np.float32)
    return np.concatenate([b0, b0, b1, b3, b1, b3, b2, gap, b2], axis=1)


def shard_inputs(x, w_qkv, w_out, b_out):
    """Full inputs -> list of 8 per-core input dicts (weights/x in bf16)."""
    import ml_dtypes
    bf16 = ml_dtypes.bfloat16
    x = np.asarray(x, dtype=np.float32)
    w_qkv = np.asarray(w_qkv, dtype=np.float32)
    w_out = np.asarray(w_out, dtype=np.float32)
    b_out = np.asarray(b_out, dtype=np.float32)
    masks = make_masks().astype(bf16)
    in_maps = []
    for c in range(N_CORES):
        b, hg = c // 2, c % 2
        h0 = hg * HPC
        csl = slice(h0 * D, (h0 + HPC) * D)
        im = {
            "xT": np.ascontiguousarray(x[b].T).astype(bf16),
            "wqT": np.ascontiguousarray(w_qkv[0 * C:1 * C][csl].T).astype(bf16),
            "wkT": np.ascontiguousarray(w_qkv[1 * C:2 * C][csl].T).astype(bf16),
            "wvT": np.ascontiguousarray(w_qkv[2 * C:3 * C][csl].T).astype(bf16),
            "woT": np.ascontiguousarray(w_out[:, csl].T).astype(bf16),
            "masks": masks,
        }
        in_maps.append(im)
    return in_maps


def gather_outputs(results, b_out=None):
    """8 per-core {'yT': [C,T]} -> full [B,T,C] (+ output bias)."""
    y = np.empty((B, T, C), np.float32)
    for b in range(B):
        acc = (results[2 * b]["yT"].astype(np.float32)
               + results[2 * b + 1]["yT"].astype(np.float32))
        y[b] = acc.T
    if b_out is not None:
        y += np.asarray(b_out, dtype=np.float32)[None, None, :]
    return y


def kernel(**inputs):
    from concourse.bass_utils import run_bass_kernel_spmd
    if "nc" not in _CACHE:
        _CACHE["nc"] = build_nc()
    nc = _CACHE["nc"]
    in_maps = shard_inputs(inputs["x"], inputs["w_qkv"],
                           inputs["w_out"], inputs["b_out"])
    res = run_bass_kernel_spmd(nc, in_maps, list(range(N_CORES)))
    return gather_outputs(res.results, inputs["b_out"])

